# revision 6
# baseline (speedup 1.0000x reference)
"""Trainium2 Bass kernel for nn_GPT5Block (MoE routing block), 8-core expert-parallel.

Contract: kernel(**inputs) takes FULL unsharded inputs (numpy, f32), returns FULL
[4,1024,512] f32 output. Internally: expert-parallel across 8 NeuronCores
(2 experts/core), dense all-expert math, ReduceScatter of the combine partials,
then per-token-shard shared-expert + FF on each core.

Layout strategy per core:
  - LayerNorm in natural [tok,D] layout (bn_stats), PE-transpose to hT [D,tok] bf16.
  - Router in f32 (exact top-2 selection): per-core COLUMN-PERMUTED router weights so
    the core's 2 local experts are always gate columns 0,1 (softmax/top-k are
    permutation invariant) -> pure SPMD program, no partition_id needed.
  - Expert chain: mm1 (w1 stationary) -> ehT [DH,tok] transposed; silu+bias fused on
    PSUM eviction; mm2 (ehT stationary) -> ey natural [tok,D]; comb scaling is then a
    per-partition scalar multiply.
  - mix partials ReduceScattered (bf16) over the 8 cores -> each core gets a
    512-token shard; stage 2 (shared expert, ff, residual) runs on the shard only.
"""

import os
import sys

for _p in ("/opt/trn_rl_repo",):
    if _p not in sys.path and os.path.isdir(_p):
        sys.path.insert(0, _p)

from contextlib import ExitStack

import numpy as np

import concourse.bass as bass
import concourse.mybir as mybir
import concourse.tile as tile
from concourse import bacc
from concourse.masks import make_identity
from concourse.bass_utils import run_bass_kernel_spmd

F32 = mybir.dt.float32
BF16 = mybir.dt.bfloat16
AF = mybir.ActivationFunctionType
ALU = mybir.AluOpType
AX = mybir.AxisListType

N_CORES = 8
B, S, D = 4, 1024, 512
T = B * S                  # 4096 tokens
E = 16
E_LOC = E // N_CORES       # 2 experts per core
DH = 4 * D                 # 2048
DS = 2 * D                 # 1024
TOP_K = 2
TEMP = 0.7
SHARED_SCALE = 0.25
EPS = 1e-5

KD = D // 128              # 4  (D chunks)
KH = DH // 128             # 16 (DH chunks)
KS = DS // 128             # 8  (DS chunks)
BLK = 512                  # token block
NBLK = T // BLK            # 8
TSH = T // N_CORES         # 512 tokens per core shard
BIG = 1.0e30


def _bc(ap: bass.AP, p: int = 128) -> bass.AP:
    """Broadcast a 1-D (or row) DRAM AP across p partitions (stride-0 DMA)."""
    return bass.AP(tensor=ap.tensor, offset=ap.offset, ap=[[0, p]] + list(ap.ap))


def build_graph():
    nc = bacc.Bacc("TRN2", target_bir_lowering=False, debug=False,
                   num_devices=N_CORES)

    def din(name, shape):
        return nc.dram_tensor(name, shape, F32, kind="ExternalInput")

    x = din("x", [T, D])
    x_sh_in = din("x_shard", [TSH, D])
    ln_g = din("ln_g", [D])
    ln_b = din("ln_b", [D])
    rw = din("rw", [D, E])        # column-permuted per core
    rb = din("rb", [E])
    ew1 = din("ew1", [E_LOC, D, DH])
    eb1 = din("eb1", [E_LOC, DH])
    ew2 = din("ew2", [E_LOC, DH, D])
    eb2 = din("eb2", [E_LOC, D])
    sw1 = din("sw1", [D, DS])
    sb1 = din("sb1", [DS])
    sw2 = din("sw2", [DS, D])
    sb2 = din("sb2", [D])
    fg = din("fg", [D])
    fb = din("fb", [D])
    fw1 = din("fw1", [D, DH])
    fb1 = din("fb1", [DH])
    fw2 = din("fw2", [DH, D])
    fb2 = din("fb2", [D])
    out = nc.dram_tensor("out", [TSH, D], F32, kind="ExternalOutput")

    with tile.TileContext(nc) as tc, ExitStack() as top:
        # ---------------- pools ----------------
        const = top.enter_context(tc.tile_pool(name="const", bufs=1))
        dram = top.enter_context(tc.tile_pool(name="dram", bufs=1, space="DRAM"))
        ps_tr = top.enter_context(tc.tile_pool(name="ps_tr", bufs=2, space="PSUM"))
        ps_gate = top.enter_context(tc.tile_pool(name="ps_gate", bufs=2, space="PSUM"))
        ps_eh = top.enter_context(tc.tile_pool(name="ps_eh", bufs=2, space="PSUM"))
        ps_ey = top.enter_context(tc.tile_pool(name="ps_ey", bufs=2, space="PSUM"))
        nat = top.enter_context(tc.tile_pool(name="nat", bufs=3))
        stats = top.enter_context(tc.tile_pool(name="stats", bufs=4))
        smalls = top.enter_context(tc.tile_pool(name="smalls", bufs=4))

        # ---------------- constants ----------------
        ident = const.tile([128, 128], F32)
        make_identity(nc, ident)
        eps_t = const.tile([128, 1], F32)
        nc.vector.memset(eps_t[:], EPS)
        g_bc = const.tile([128, D], F32)
        nc.sync.dma_start(out=g_bc[:], in_=_bc(ln_g[:]))
        b_bc = const.tile([128, D], F32)
        nc.sync.dma_start(out=b_bc[:], in_=_bc(ln_b[:]))
        rb_bc = const.tile([128, E], F32)
        nc.sync.dma_start(out=rb_bc[:], in_=_bc(rb[:]))
        fg_bc = const.tile([128, D], F32)
        nc.sync.dma_start(out=fg_bc[:], in_=_bc(fg[:]))
        fb_bc = const.tile([128, D], F32)
        nc.sync.dma_start(out=fb_bc[:], in_=_bc(fb[:]))
        sb2_bc = const.tile([128, D], F32)
        nc.sync.dma_start(out=sb2_bc[:], in_=_bc(sb2[:]))
        fb2_bc = const.tile([128, D], F32)
        nc.sync.dma_start(out=fb2_bc[:], in_=_bc(fb2[:]))
        eb2_bc = const.tile([128, E_LOC, D], F32)
        nc.sync.dma_start(out=eb2_bc[:], in_=_bc(eb2[:, :]))
        # per-partition bias layouts (feature dim on partitions)
        eb1_t = const.tile([128, E_LOC, KH], F32)
        nc.sync.dma_start(
            out=eb1_t[:],
            in_=bass.AP(tensor=eb1, offset=0,
                        ap=[[1, 128], [DH, E_LOC], [128, KH]]))
        sb1_t = const.tile([128, KS], F32)
        nc.sync.dma_start(
            out=sb1_t[:],
            in_=bass.AP(tensor=sb1, offset=0, ap=[[1, 128], [128, KS]]))
        fb1_t = const.tile([128, KH], F32)
        nc.sync.dma_start(
            out=fb1_t[:],
            in_=bass.AP(tensor=fb1, offset=0, ap=[[1, 128], [128, KH]]))
        rw_t = const.tile([128, KD, E], F32)
        nc.sync.dma_start(
            out=rw_t[:],
            in_=bass.AP(tensor=rw, offset=0,
                        ap=[[E, 128], [128 * E, KD], [1, E]]))

        # expert weights, bf16 (gpsimd DMA casts f32->bf16)
        wts = top.enter_context(tc.tile_pool(name="wts", bufs=1))
        ew1_t = wts.tile([128, E_LOC, KD, DH], BF16)
        nc.gpsimd.dma_start(
            out=ew1_t[:],
            in_=bass.AP(tensor=ew1, offset=0,
                        ap=[[DH, 128], [D * DH, E_LOC], [128 * DH, KD], [1, DH]]))
        ew2_t = wts.tile([128, E_LOC, KH, D], BF16)
        nc.gpsimd.dma_start(
            out=ew2_t[:],
            in_=bass.AP(tensor=ew2, offset=0,
                        ap=[[D, 128], [DH * D, E_LOC], [128 * D, KH], [1, D]]))

        # DRAM bounce for the collective
        mix_dram = dram.tile([T, D], BF16)
        rs_out = dram.tile([TSH, D], BF16)

        # ---------------- stage 1 ----------------
        if True:
            hT_pool = top.enter_context(tc.tile_pool(name="hT", bufs=2))
            hTf = top.enter_context(tc.tile_pool(name="hTf", bufs=2))
            ehp = top.enter_context(tc.tile_pool(name="ehp", bufs=2))
            mixp = top.enter_context(tc.tile_pool(name="mixp", bufs=2))
            combp = top.enter_context(tc.tile_pool(name="combp", bufs=2))
            eytmp = top.enter_context(tc.tile_pool(name="eytmp", bufs=3))

            for b in range(NBLK):
                hT_b = hT_pool.tile([128, KD, BLK], BF16, tag="hT")
                hTf_b = hTf.tile([128, KD, BLK], F32, tag="hTf")
                comb_b = combp.tile([128, BLK // 128, E], F32, tag="comb")
                for j in range(BLK // 128):
                    i = b * (BLK // 128) + j
                    x_t = nat.tile([128, D], F32, tag="x_nat")
                    nc.sync.dma_start(out=x_t[:], in_=x[i * 128:(i + 1) * 128, :])
                    st = stats.tile([128, 6], F32, tag="st")
                    nc.vector.bn_stats(out=st[:], in_=x_t[:])
                    mv = stats.tile([128, 2], F32, tag="mv")
                    nc.vector.bn_aggr(out=mv[:], in_=st[:])
                    rstd = stats.tile([128, 1], F32, tag="rstd")
                    nc.scalar.activation(out=rstd[:], in_=mv[:, 1:2],
                                         func=AF.Sqrt, bias=eps_t[:])
                    nc.vector.reciprocal(rstd[:], rstd[:])
                    h_t = nat.tile([128, D], F32, tag="h_nat")
                    nc.vector.tensor_scalar(out=h_t[:], in0=x_t[:],
                                            scalar1=mv[:, 0:1], scalar2=rstd[:],
                                            op0=ALU.subtract, op1=ALU.mult)
                    nc.vector.tensor_mul(h_t[:], h_t[:], g_bc[:])
                    nc.vector.tensor_add(h_t[:], h_t[:], b_bc[:])
                    # transpose 4x [128,128] -> psum, evict to bf16 hT + f32 hTf
                    ps_t = ps_tr.tile([128, D], F32, tag="tr")
                    for k in range(KD):
                        nc.tensor.transpose(ps_t[:, k * 128:(k + 1) * 128],
                                            h_t[:, k * 128:(k + 1) * 128],
                                            ident[:])
                    ps_v = ps_t[:].rearrange("p (k t) -> p k t", k=KD)
                    nc.scalar.copy(hT_b[:, :, j * 128:(j + 1) * 128], ps_v)
                    nc.vector.tensor_copy(hTf_b[:, :, j * 128:(j + 1) * 128], ps_v)
                    # router (f32, exact selection)
                    g_ps = ps_gate.tile([128, E], F32, tag="gate")
                    for k in range(KD):
                        nc.tensor.matmul(g_ps[:],
                                         hTf_b[:, k, j * 128:(j + 1) * 128],
                                         rw_t[:, k, :],
                                         start=(k == 0), stop=(k == KD - 1))
                    gate = smalls.tile([128, E], F32, tag="gate_sb")
                    nc.vector.tensor_add(gate[:], g_ps[:], rb_bc[:])
                    m1 = stats.tile([128, 1], F32, tag="m1")
                    nc.vector.reduce_max(m1[:], gate[:], axis=AX.X, negate=True)
                    m1s = stats.tile([128, 1], F32, tag="m1s")
                    nc.vector.tensor_scalar_mul(m1s[:], m1[:], 1.0 / TEMP)
                    prob = smalls.tile([128, E], F32, tag="prob")
                    nc.scalar.activation(out=prob[:], in_=gate[:], func=AF.Exp,
                                         scale=1.0 / TEMP, bias=m1s[:])
                    ssum = stats.tile([128, 1], F32, tag="ssum")
                    nc.vector.reduce_sum(ssum[:], prob[:], axis=AX.X)
                    rsum = stats.tile([128, 1], F32, tag="rsum")
                    nc.vector.reciprocal(rsum[:], ssum[:])
                    nc.vector.tensor_scalar_mul(prob[:], prob[:], rsum[:])
                    mg = stats.tile([128, 1], F32, tag="mg")
                    nc.vector.reduce_max(mg[:], prob[:], axis=AX.X)
                    is1 = smalls.tile([128, E], F32, tag="is1")
                    nc.vector.tensor_scalar(out=is1[:], in0=prob[:],
                                            scalar1=mg[:], scalar2=-BIG,
                                            op0=ALU.is_ge, op1=ALU.mult)
                    g2 = smalls.tile([128, E], F32, tag="g2")
                    nc.vector.tensor_add(g2[:], prob[:], is1[:])
                    m2 = stats.tile([128, 1], F32, tag="m2")
                    nc.vector.reduce_max(m2[:], g2[:], axis=AX.X)
                    is2 = smalls.tile([128, E], F32, tag="is2")
                    nc.vector.tensor_scalar(out=is2[:], in0=prob[:],
                                            scalar1=m2[:], scalar2=None,
                                            op0=ALU.is_ge)
                    nc.vector.tensor_mul(comb_b[:, j, :], prob[:], is2[:])

                # experts (dense) for this token block
                mix_b = mixp.tile([128, BLK // 128, D], F32, tag="mix")
                for e in range(E_LOC):
                    ehT = ehp.tile([128, KH, BLK], BF16, tag="ehT")
                    for m in range(KH):
                        ps1 = ps_eh.tile([128, BLK], F32, tag="eh")
                        for k in range(KD):
                            nc.tensor.matmul(ps1[:],
                                             ew1_t[:, e, k, m * 128:(m + 1) * 128],
                                             hT_b[:, k, :],
                                             start=(k == 0), stop=(k == KD - 1))
                        nc.scalar.activation(out=ehT[:, m, :], in_=ps1[:],
                                             func=AF.Silu,
                                             bias=eb1_t[:, e, m:m + 1])
                    for ms in range(BLK // 128):
                        ps2 = ps_ey.tile([128, D], F32, tag="ey")
                        for k in range(KH):
                            nc.tensor.matmul(ps2[:],
                                             ehT[:, k, ms * 128:(ms + 1) * 128],
                                             ew2_t[:, e, k, :],
                                             start=(k == 0), stop=(k == KH - 1))
                        t0 = eytmp.tile([128, D], F32, tag="eyt")
                        nc.vector.tensor_add(t0[:], ps2[:], eb2_bc[:, e, :])
                        if e == 0:
                            nc.vector.tensor_scalar_mul(
                                mix_b[:, ms, :], t0[:], comb_b[:, ms, 0:1])
                        else:
                            t1 = eytmp.tile([128, D], F32, tag="eyt2")
                            nc.vector.tensor_scalar_mul(
                                t1[:], t0[:], comb_b[:, ms, e:e + 1])
                            nc.vector.tensor_add(
                                mix_b[:, ms, :], mix_b[:, ms, :], t1[:])
                # cast f32 -> bf16 on the way out (gpsimd DMA casts)
                nc.gpsimd.dma_start(
                    out=mix_dram[b * BLK:(b + 1) * BLK, :].rearrange(
                        "(ms p) d -> p ms d", p=128),
                    in_=mix_b[:])

        # ---------------- collective ----------------
        nc.gpsimd.collective_compute(
            "ReduceScatter", ALU.add,
            replica_groups=[list(range(N_CORES))],
            ins=[mix_dram[:].opt()],
            outs=[rs_out[:].opt()],
        )

        # ---------------- stage 2 (on 512-token shard) ----------------
        with ExitStack() as s2:
            s2w = s2.enter_context(tc.tile_pool(name="s2w", bufs=3))

            MS = TSH // 128  # 4
            # reuse stage-1 pool slots via tags (mix: x_sh+moe; hTf: mix_sh+w1s;
            # hT: hT_sh+fhT; ehT: ehsT/ffhT/fw1)
            x_sh = mixp.tile([128, MS, D], F32, tag="mix", name="x_sh")
            nc.sync.dma_start(
                out=x_sh[:],
                in_=x_sh_in[:].rearrange("(ms p) d -> p ms d", p=128))
            mix_sh = hTf.tile([128, MS, D], F32, tag="hTf", name="mix_sh")
            nc.gpsimd.dma_start(   # bf16 -> f32 cast
                out=mix_sh[:],
                in_=rs_out[:].rearrange("(ms p) d -> p ms d", p=128))

            # shared expert input: LN of x_shard (transposed, bf16)
            hT_sh = hT_pool.tile([128, KD, TSH], BF16, tag="hT", name="hT_sh")
            for j2 in range(MS):
                st = stats.tile([128, 6], F32, tag="st", name="st")
                nc.vector.bn_stats(out=st[:], in_=x_sh[:, j2, :])
                mv = stats.tile([128, 2], F32, tag="mv", name="mv")
                nc.vector.bn_aggr(out=mv[:], in_=st[:])
                rstd = stats.tile([128, 1], F32, tag="rstd", name="rstd")
                nc.scalar.activation(out=rstd[:], in_=mv[:, 1:2],
                                     func=AF.Sqrt, bias=eps_t[:])
                nc.vector.reciprocal(rstd[:], rstd[:])
                h_t = nat.tile([128, D], F32, tag="h_nat", name="h_t")
                nc.vector.tensor_scalar(out=h_t[:], in0=x_sh[:, j2, :],
                                        scalar1=mv[:, 0:1], scalar2=rstd[:],
                                        op0=ALU.subtract, op1=ALU.mult)
                nc.vector.tensor_mul(h_t[:], h_t[:], g_bc[:])
                nc.vector.tensor_add(h_t[:], h_t[:], b_bc[:])
                ps_t = ps_tr.tile([128, D], F32, tag="tr", name="ps_t")
                for k in range(KD):
                    nc.tensor.transpose(ps_t[:, k * 128:(k + 1) * 128],
                                        h_t[:, k * 128:(k + 1) * 128], ident[:])
                nc.scalar.copy(hT_sh[:, :, j2 * 128:(j2 + 1) * 128],
                               ps_t[:].rearrange("p (k t) -> p k t", k=KD))

            # shared expert mm1: ehsT [DS, TSH]
            sw1_t = hTf.tile([128, KD, DS], BF16, tag="hTf", name="sw1_t")
            nc.gpsimd.dma_start(
                out=sw1_t[:],
                in_=bass.AP(tensor=sw1, offset=0,
                            ap=[[DS, 128], [128 * DS, KD], [1, DS]]))
            ehsT = ehp.tile([128, KS, TSH], BF16, tag="ehT", name="ehsT")
            for m in range(KS):
                ps1 = ps_eh.tile([128, TSH], F32, tag="eh", name="ps1")
                for k in range(KD):
                    nc.tensor.matmul(ps1[:], sw1_t[:, k, m * 128:(m + 1) * 128],
                                     hT_sh[:, k, :],
                                     start=(k == 0), stop=(k == KD - 1))
                nc.scalar.activation(out=ehsT[:, m, :], in_=ps1[:], func=AF.Silu,
                                     bias=sb1_t[:, m:m + 1])
            # shared mm2 (k-outer, 4 psum banks) -> natural, *0.25 +sb2*0.25; +mix
            ps2s = [ps_ey.tile([128, D], F32, tag="ey", name=f"ps2s{ms}")
                    if ms < 2 else
                    ps_gate.tile([128, D], F32, tag="gate", name=f"ps2s{ms}")
                    for ms in range(MS)]
            for k in range(KS):
                w2k = s2w.tile([128, D], BF16, tag="s2wk", name="w2k")
                nc.gpsimd.dma_start(
                    out=w2k[:], in_=sw2[k * 128:(k + 1) * 128, :])
                for ms in range(MS):
                    nc.tensor.matmul(ps2s[ms][:],
                                     ehsT[:, k, ms * 128:(ms + 1) * 128],
                                     w2k[:],
                                     start=(k == 0), stop=(k == KS - 1))
            moe = mixp.tile([128, MS, D], F32, tag="mix", name="moe")
            for ms in range(MS):
                sh_t = nat.tile([128, D], F32, tag="sh_t", name="sh_t")
                nc.vector.tensor_add(sh_t[:], ps2s[ms][:], sb2_bc[:])
                nc.vector.tensor_scalar(out=sh_t[:], in0=sh_t[:],
                                        scalar1=SHARED_SCALE, scalar2=None,
                                        op0=ALU.mult)
                nc.vector.tensor_add(moe[:, ms, :], sh_t[:], mix_sh[:, ms, :])

            # ff LN -> fhT
            fhT = hT_pool.tile([128, KD, TSH], BF16, tag="hT", name="fhT")
            for j2 in range(MS):
                st = stats.tile([128, 6], F32, tag="st", name="st")
                nc.vector.bn_stats(out=st[:], in_=moe[:, j2, :])
                mv = stats.tile([128, 2], F32, tag="mv", name="mv")
                nc.vector.bn_aggr(out=mv[:], in_=st[:])
                rstd = stats.tile([128, 1], F32, tag="rstd", name="rstd")
                nc.scalar.activation(out=rstd[:], in_=mv[:, 1:2],
                                     func=AF.Sqrt, bias=eps_t[:])
                nc.vector.reciprocal(rstd[:], rstd[:])
                h_t = nat.tile([128, D], F32, tag="h_nat", name="h_t")
                nc.vector.tensor_scalar(out=h_t[:], in0=moe[:, j2, :],
                                        scalar1=mv[:, 0:1], scalar2=rstd[:],
                                        op0=ALU.subtract, op1=ALU.mult)
                nc.vector.tensor_mul(h_t[:], h_t[:], fg_bc[:])
                nc.vector.tensor_add(h_t[:], h_t[:], fb_bc[:])
                ps_t = ps_tr.tile([128, D], F32, tag="tr", name="ps_t")
                for k in range(KD):
                    nc.tensor.transpose(ps_t[:, k * 128:(k + 1) * 128],
                                        h_t[:, k * 128:(k + 1) * 128], ident[:])
                nc.scalar.copy(fhT[:, :, j2 * 128:(j2 + 1) * 128],
                               ps_t[:].rearrange("p (k t) -> p k t", k=KD))

            # ff mm1: ffhT [DH, TSH]
            fw1_t = ehp.tile([128, KD, DH], BF16, tag="ehT", name="fw1_t")
            nc.gpsimd.dma_start(
                out=fw1_t[:],
                in_=bass.AP(tensor=fw1, offset=0,
                            ap=[[DH, 128], [128 * DH, KD], [1, DH]]))
            ffhT = ehp.tile([128, KH, TSH], BF16, tag="ehT", name="ffhT")
            for m in range(KH):
                ps1 = ps_eh.tile([128, TSH], F32, tag="eh", name="ps1")
                for k in range(KD):
                    nc.tensor.matmul(ps1[:], fw1_t[:, k, m * 128:(m + 1) * 128],
                                     fhT[:, k, :],
                                     start=(k == 0), stop=(k == KD - 1))
                nc.scalar.activation(out=ffhT[:, m, :], in_=ps1[:], func=AF.Silu,
                                     bias=fb1_t[:, m:m + 1])
            # ff mm2 (k-outer, 4 psum banks) -> natural; out = x + moe + ff
            ps3s = [ps_ey.tile([128, D], F32, tag="ey", name=f"ps3s{ms}")
                    if ms < 2 else
                    ps_gate.tile([128, D], F32, tag="gate", name=f"ps3s{ms}")
                    for ms in range(MS)]
            for k in range(KH):
                w2k = s2w.tile([128, D], BF16, tag="s2wk", name="w2k")
                nc.gpsimd.dma_start(
                    out=w2k[:], in_=fw2[k * 128:(k + 1) * 128, :])
                for ms in range(MS):
                    nc.tensor.matmul(ps3s[ms][:],
                                     ffhT[:, k, ms * 128:(ms + 1) * 128],
                                     w2k[:],
                                     start=(k == 0), stop=(k == KH - 1))
            for ms in range(MS):
                o_t = nat.tile([128, D], F32, tag="o_t", name="o_t")
                nc.vector.tensor_add(o_t[:], ps3s[ms][:], fb2_bc[:])
                nc.vector.tensor_add(o_t[:], o_t[:], moe[:, ms, :])
                nc.vector.tensor_add(o_t[:], o_t[:], x_sh[:, ms, :])
                nc.sync.dma_start(
                    out=out[ms * 128:(ms + 1) * 128, :], in_=o_t[:])

    nc.compile()
    return nc


_GRAPH = None


def _get_graph():
    global _GRAPH
    if _GRAPH is None:
        _GRAPH = build_graph()
    return _GRAPH


def _make_in_maps(inputs):
    x = np.ascontiguousarray(inputs["x"].reshape(T, D), dtype=np.float32)
    rw_full = np.asarray(inputs["router_w"], dtype=np.float32)
    rb_full = np.asarray(inputs["router_b"], dtype=np.float32)
    in_maps = []
    for i in range(N_CORES):
        lo, hi = i * E_LOC, (i + 1) * E_LOC
        perm = list(range(lo, hi)) + [e for e in range(E) if not lo <= e < hi]
        m = {
            "x": x,
            "x_shard": np.ascontiguousarray(x[i * TSH:(i + 1) * TSH, :]),
            "ln_g": np.asarray(inputs["ln_in_g"], np.float32),
            "ln_b": np.asarray(inputs["ln_in_b"], np.float32),
            "rw": np.ascontiguousarray(rw_full[:, perm]),
            "rb": np.ascontiguousarray(rb_full[perm]),
            "ew1": np.ascontiguousarray(inputs["ew1"][lo:hi], np.float32),
            "eb1": np.ascontiguousarray(inputs["eb1"][lo:hi], np.float32),
            "ew2": np.ascontiguousarray(inputs["ew2"][lo:hi], np.float32),
            "eb2": np.ascontiguousarray(inputs["eb2"][lo:hi], np.float32),
            "sw1": np.asarray(inputs["sw1"], np.float32),
            "sb1": np.asarray(inputs["sb1"], np.float32),
            "sw2": np.asarray(inputs["sw2"], np.float32),
            "sb2": np.asarray(inputs["sb2"], np.float32),
            "fg": np.asarray(inputs["ff_ln_g"], np.float32),
            "fb": np.asarray(inputs["ff_ln_b"], np.float32),
            "fw1": np.asarray(inputs["ff_w1"], np.float32),
            "fb1": np.asarray(inputs["ff_b1"], np.float32),
            "fw2": np.asarray(inputs["ff_w2"], np.float32),
            "fb2": np.asarray(inputs["ff_b2"], np.float32),
        }
        in_maps.append(m)
    return in_maps


def kernel_ex(trace=False, **inputs):
    nc = _get_graph()
    in_maps = _make_in_maps(inputs)
    res = run_bass_kernel_spmd(nc, in_maps, list(range(N_CORES)), trace=trace)
    full = np.empty((T, D), dtype=np.float32)
    for i in range(N_CORES):
        full[i * TSH:(i + 1) * TSH, :] = res.results[i]["out"]
    return full.reshape(B, S, D), res


def kernel(**inputs):
    out, _ = kernel_ex(**inputs)
    return out


# revision 7
# speedup vs baseline: 1.0486x; 1.0486x over previous
"""Trainium2 Bass kernel for nn_GPT5Block (MoE routing block), 8-core expert-parallel.

Contract: kernel(**inputs) takes FULL unsharded inputs (numpy, f32), returns FULL
[4,1024,512] f32 output. Internally: expert-parallel across 8 NeuronCores
(2 experts/core), dense all-expert math, chunked ReduceScatter of the combine
partials overlapped with expert compute, then per-token-shard FF on each core.

Layout strategy per core:
  - LayerNorm in natural [tok,D] layout (bn_stats), PE-transpose to hT [D,tok] bf16.
  - Router in f32 (exact top-2 selection): per-core COLUMN-PERMUTED router weights so
    the core's 2 local experts are always gate columns 0,1 (softmax/top-k are
    permutation invariant) -> pure SPMD program, no partition_id needed.
  - Expert chain: mm1 (w1 stationary) -> ehT [DH,tok] transposed; silu+bias fused on
    PSUM eviction; mm2 (ehT stationary) -> ey natural [tok,D]; comb scaling is then a
    per-partition scalar multiply.
  - Shared expert depends only on x_shard -> computed FIRST (overlaps with weight DMA
    and warms the PE before the big expert stream).
  - mix partials ReduceScattered (bf16) in 4 chunks, each issued as soon as its 2
    token blocks are done -> comm overlaps the remaining expert compute.
  - Token shard per core (chunked RS scatter): rows {c*1024 + core*128 .. +128} for
    chunk c in 0..3 (host slices x_shard / reassembles output accordingly).
"""

import os
import sys

for _p in ("/opt/trn_rl_repo",):
    if _p not in sys.path and os.path.isdir(_p):
        sys.path.insert(0, _p)

from contextlib import ExitStack

import numpy as np

import concourse.bass as bass
import concourse.mybir as mybir
import concourse.tile as tile
from concourse import bacc
from concourse.masks import make_identity
from concourse.bass_utils import run_bass_kernel_spmd

F32 = mybir.dt.float32
BF16 = mybir.dt.bfloat16
AF = mybir.ActivationFunctionType
ALU = mybir.AluOpType
AX = mybir.AxisListType

N_CORES = 8
B, S, D = 4, 1024, 512
T = B * S                  # 4096 tokens
E = 16
E_LOC = E // N_CORES       # 2 experts per core
DH = 4 * D                 # 2048
DS = 2 * D                 # 1024
TEMP = 0.7
SHARED_SCALE = 0.25
EPS = 1e-5

KD = D // 128              # 4  (D chunks)
KH = DH // 128             # 16 (DH chunks)
KS = DS // 128             # 8  (DS chunks)
BLK = 512                  # token block
NBLK = T // BLK            # 8
TSH = T // N_CORES         # 512 tokens per core shard
NCH = 4                    # RS chunks
CHT = T // NCH             # 1024 tokens per RS chunk
MS = TSH // 128            # 4 (= NCH)
BIG = 1.0e30


def _bc(ap: bass.AP, p: int = 128) -> bass.AP:
    """Broadcast a 1-D (or row) DRAM AP across p partitions (stride-0 DMA)."""
    return bass.AP(tensor=ap.tensor, offset=ap.offset, ap=[[0, p]] + list(ap.ap))


def _shard_rows(core):
    """Global token rows owned by `core` under the chunked reduce-scatter."""
    rows = []
    for c in range(NCH):
        s = c * CHT + core * (CHT // N_CORES)
        rows.append((s, s + CHT // N_CORES))
    return rows


def build_graph():
    nc = bacc.Bacc("TRN2", target_bir_lowering=False, debug=False,
                   num_devices=N_CORES)

    def din(name, shape):
        return nc.dram_tensor(name, shape, F32, kind="ExternalInput")

    x = din("x", [T, D])
    x_sh_in = din("x_shard", [TSH, D])
    ln_g = din("ln_g", [D])
    ln_b = din("ln_b", [D])
    rw = din("rw", [D, E])        # column-permuted per core
    rb = din("rb", [E])
    ew1 = din("ew1", [E_LOC, D, DH])
    eb1 = din("eb1", [E_LOC, DH])
    ew2 = din("ew2", [E_LOC, DH, D])
    eb2 = din("eb2", [E_LOC, D])
    sw1 = din("sw1", [D, DS])
    sb1 = din("sb1", [DS])
    sw2 = din("sw2", [DS, D])
    sb2 = din("sb2", [D])
    fg = din("fg", [D])
    fb = din("fb", [D])
    fw1 = din("fw1", [D, DH])
    fb1 = din("fb1", [DH])
    fw2 = din("fw2", [DH, D])
    fb2 = din("fb2", [D])
    out = nc.dram_tensor("out", [TSH, D], F32, kind="ExternalOutput")

    with tile.TileContext(nc) as tc, ExitStack() as top:
        # ---------------- pools ----------------
        const = top.enter_context(tc.tile_pool(name="const", bufs=1))
        dram = top.enter_context(tc.tile_pool(name="dram", bufs=1, space="DRAM"))
        ps_tr = top.enter_context(tc.tile_pool(name="ps_tr", bufs=2, space="PSUM"))
        ps_gate = top.enter_context(tc.tile_pool(name="ps_gate", bufs=2, space="PSUM"))
        ps_eh = top.enter_context(tc.tile_pool(name="ps_eh", bufs=2, space="PSUM"))
        ps_ey = top.enter_context(tc.tile_pool(name="ps_ey", bufs=2, space="PSUM"))
        nat = top.enter_context(tc.tile_pool(name="nat", bufs=3))
        stats = top.enter_context(tc.tile_pool(name="stats", bufs=4))
        smalls = top.enter_context(tc.tile_pool(name="smalls", bufs=4))
        hT_pool = top.enter_context(tc.tile_pool(name="hT", bufs=2))
        hTf = top.enter_context(tc.tile_pool(name="hTf", bufs=2))
        ehp = top.enter_context(tc.tile_pool(name="ehp", bufs=2))
        mixp = top.enter_context(tc.tile_pool(name="mixp", bufs=4))
        combp = top.enter_context(tc.tile_pool(name="combp", bufs=2))
        eytmp = top.enter_context(tc.tile_pool(name="eytmp", bufs=3))
        s2w = top.enter_context(tc.tile_pool(name="s2w", bufs=3))

        # ---------------- constants ----------------
        ident = const.tile([128, 128], F32)
        make_identity(nc, ident)
        eps_t = const.tile([128, 1], F32)
        nc.vector.memset(eps_t[:], EPS)
        g_bc = const.tile([128, D], F32)
        nc.sync.dma_start(out=g_bc[:], in_=_bc(ln_g[:]))
        b_bc = const.tile([128, D], F32)
        nc.sync.dma_start(out=b_bc[:], in_=_bc(ln_b[:]))
        rb_bc = const.tile([128, E], F32)
        nc.sync.dma_start(out=rb_bc[:], in_=_bc(rb[:]))
        fg_bc = const.tile([128, D], F32)
        nc.sync.dma_start(out=fg_bc[:], in_=_bc(fg[:]))
        fb_bc = const.tile([128, D], F32)
        nc.sync.dma_start(out=fb_bc[:], in_=_bc(fb[:]))
        sb2_bc = const.tile([128, D], F32)
        nc.sync.dma_start(out=sb2_bc[:], in_=_bc(sb2[:]))
        fb2_bc = const.tile([128, D], F32)
        nc.sync.dma_start(out=fb2_bc[:], in_=_bc(fb2[:]))
        eb2_bc = const.tile([128, E_LOC, D], F32)
        nc.sync.dma_start(out=eb2_bc[:], in_=_bc(eb2[:, :]))
        eb1_t = const.tile([128, E_LOC, KH], F32)
        nc.sync.dma_start(
            out=eb1_t[:],
            in_=bass.AP(tensor=eb1, offset=0,
                        ap=[[1, 128], [DH, E_LOC], [128, KH]]))
        sb1_t = const.tile([128, KS], F32)
        nc.sync.dma_start(
            out=sb1_t[:],
            in_=bass.AP(tensor=sb1, offset=0, ap=[[1, 128], [128, KS]]))
        fb1_t = const.tile([128, KH], F32)
        nc.sync.dma_start(
            out=fb1_t[:],
            in_=bass.AP(tensor=fb1, offset=0, ap=[[1, 128], [128, KH]]))
        rw_t = const.tile([128, KD, E], F32)
        nc.sync.dma_start(
            out=rw_t[:],
            in_=bass.AP(tensor=rw, offset=0,
                        ap=[[E, 128], [128 * E, KD], [1, E]]))

        # expert weights, bf16 (gpsimd DMA casts f32->bf16)
        wts = top.enter_context(tc.tile_pool(name="wts", bufs=1))
        ew1_t = wts.tile([128, E_LOC, KD, DH], BF16)
        nc.gpsimd.dma_start(
            out=ew1_t[:],
            in_=bass.AP(tensor=ew1, offset=0,
                        ap=[[DH, 128], [D * DH, E_LOC], [128 * DH, KD], [1, DH]]))
        ew2_t = wts.tile([128, E_LOC, KH, D], BF16)
        nc.gpsimd.dma_start(
            out=ew2_t[:],
            in_=bass.AP(tensor=ew2, offset=0,
                        ap=[[D, 128], [DH * D, E_LOC], [128 * D, KH], [1, D]]))

        # DRAM bounce for the collective
        mix_dram = dram.tile([T, D], BF16)
        rs_out = dram.tile([TSH, D], BF16)

        def layernorm_tile(src_ap, gamma, beta, name_sfx=""):
            """LN along free dim of a [128, D] f32 AP -> new f32 tile."""
            st = stats.tile([128, 6], F32, tag="st", name="st" + name_sfx)
            nc.vector.bn_stats(out=st[:], in_=src_ap)
            mv = stats.tile([128, 2], F32, tag="mv", name="mv" + name_sfx)
            nc.vector.bn_aggr(out=mv[:], in_=st[:])
            rstd = stats.tile([128, 1], F32, tag="rstd", name="rstd" + name_sfx)
            nc.scalar.activation(out=rstd[:], in_=mv[:, 1:2],
                                 func=AF.Sqrt, bias=eps_t[:])
            nc.vector.reciprocal(rstd[:], rstd[:])
            h_t = nat.tile([128, D], F32, tag="h_nat", name="h_t" + name_sfx)
            nc.vector.tensor_scalar(out=h_t[:], in0=src_ap,
                                    scalar1=mv[:, 0:1], scalar2=rstd[:],
                                    op0=ALU.subtract, op1=ALU.mult)
            nc.vector.tensor_mul(h_t[:], h_t[:], gamma[:])
            nc.vector.tensor_add(h_t[:], h_t[:], beta[:])
            return h_t

        def transpose_to(h_t, dst_slices):
            """PE-transpose [128, D] f32 tile -> write [128, KD, 128] views."""
            ps_t = ps_tr.tile([128, D], F32, tag="tr", name="ps_tq")
            for k in range(KD):
                nc.tensor.transpose(ps_t[:, k * 128:(k + 1) * 128],
                                    h_t[:, k * 128:(k + 1) * 128], ident[:])
            ps_v = ps_t[:].rearrange("p (k t) -> p k t", k=KD)
            for dst, engine in dst_slices:
                if engine == "act":
                    nc.scalar.copy(dst, ps_v)
                else:
                    nc.vector.tensor_copy(dst, ps_v)

        # ============ shared expert (depends only on x_shard; runs early) ====
        x_sh = mixp.tile([128, MS, D], F32, tag="mix", name="x_sh")
        nc.sync.dma_start(
            out=x_sh[:],
            in_=x_sh_in[:].rearrange("(ms p) d -> p ms d", p=128))
        hT_sh = hT_pool.tile([128, KD, TSH], BF16, tag="hT", name="hT_sh")
        for j2 in range(MS):
            h_t = layernorm_tile(x_sh[:, j2, :], g_bc, b_bc, "_sh")
            transpose_to(h_t, [(hT_sh[:, :, j2 * 128:(j2 + 1) * 128], "act")])

        sw1_t = hTf.tile([128, KD, DS], BF16, tag="hTf", name="sw1_t")
        nc.gpsimd.dma_start(
            out=sw1_t[:],
            in_=bass.AP(tensor=sw1, offset=0,
                        ap=[[DS, 128], [128 * DS, KD], [1, DS]]))
        ehsT = ehp.tile([128, KS, TSH], BF16, tag="ehT", name="ehsT")
        for m in range(KS):
            ps1 = ps_eh.tile([128, TSH], F32, tag="eh", name="ps1s")
            for k in range(KD):
                nc.tensor.matmul(ps1[:], sw1_t[:, k, m * 128:(m + 1) * 128],
                                 hT_sh[:, k, :],
                                 start=(k == 0), stop=(k == KD - 1))
            nc.scalar.activation(out=ehsT[:, m, :], in_=ps1[:], func=AF.Silu,
                                 bias=sb1_t[:, m:m + 1])
        # shared mm2 (k-outer over 4 psum banks) -> natural; *0.25, +sb2*0.25
        ps2s = [ps_ey.tile([128, D], F32, tag="ey", name=f"ps2s{ms}")
                if ms < 2 else
                ps_gate.tile([128, D], F32, tag="gate", name=f"ps2s{ms}")
                for ms in range(MS)]
        for k in range(KS):
            w2k = s2w.tile([128, D], BF16, tag="s2wk", name="w2ks")
            nc.gpsimd.dma_start(out=w2k[:], in_=sw2[k * 128:(k + 1) * 128, :])
            for ms in range(MS):
                nc.tensor.matmul(ps2s[ms][:],
                                 ehsT[:, k, ms * 128:(ms + 1) * 128],
                                 w2k[:],
                                 start=(k == 0), stop=(k == KS - 1))
        shared_nat = mixp.tile([128, MS, D], F32, tag="mix", name="shared_nat")
        for ms in range(MS):
            nc.vector.tensor_add(shared_nat[:, ms, :], ps2s[ms][:], sb2_bc[:])
            nc.vector.tensor_scalar(out=shared_nat[:, ms, :],
                                    in0=shared_nat[:, ms, :],
                                    scalar1=SHARED_SCALE, scalar2=None,
                                    op0=ALU.mult)

        # ============ stage 1: LN + router + dense experts, per token block ==
        for b in range(NBLK):
            hT_b = hT_pool.tile([128, KD, BLK], BF16, tag="hT", name="hT_b")
            hTf_b = hTf.tile([128, KD, BLK], F32, tag="hTf", name="hTf_b")
            comb_b = combp.tile([128, BLK // 128, E], F32, tag="comb",
                                name="comb_b")
            for j in range(BLK // 128):
                i = b * (BLK // 128) + j
                x_t = nat.tile([128, D], F32, tag="x_nat", name="x_t")
                nc.sync.dma_start(out=x_t[:], in_=x[i * 128:(i + 1) * 128, :])
                h_t = layernorm_tile(x_t[:], g_bc, b_bc)
                transpose_to(h_t, [
                    (hT_b[:, :, j * 128:(j + 1) * 128], "act"),
                    (hTf_b[:, :, j * 128:(j + 1) * 128], "vec")])
                # router (f32, exact selection)
                g_ps = ps_gate.tile([128, E], F32, tag="gate", name="g_ps")
                for k in range(KD):
                    nc.tensor.matmul(g_ps[:],
                                     hTf_b[:, k, j * 128:(j + 1) * 128],
                                     rw_t[:, k, :],
                                     start=(k == 0), stop=(k == KD - 1))
                gate = smalls.tile([128, E], F32, tag="gate_sb", name="gate")
                nc.vector.tensor_add(gate[:], g_ps[:], rb_bc[:])
                m1 = stats.tile([128, 1], F32, tag="m1", name="m1")
                nc.vector.reduce_max(m1[:], gate[:], axis=AX.X, negate=True)
                m1s = stats.tile([128, 1], F32, tag="m1s", name="m1s")
                nc.vector.tensor_scalar_mul(m1s[:], m1[:], 1.0 / TEMP)
                prob = smalls.tile([128, E], F32, tag="prob", name="prob")
                nc.scalar.activation(out=prob[:], in_=gate[:], func=AF.Exp,
                                     scale=1.0 / TEMP, bias=m1s[:])
                ssum = stats.tile([128, 1], F32, tag="ssum", name="ssum")
                nc.vector.reduce_sum(ssum[:], prob[:], axis=AX.X)
                rsum = stats.tile([128, 1], F32, tag="rsum", name="rsum")
                nc.vector.reciprocal(rsum[:], ssum[:])
                nc.vector.tensor_scalar_mul(prob[:], prob[:], rsum[:])
                mg = stats.tile([128, 1], F32, tag="mg", name="mg")
                nc.vector.reduce_max(mg[:], prob[:], axis=AX.X)
                is1 = smalls.tile([128, E], F32, tag="is1", name="is1")
                nc.vector.tensor_scalar(out=is1[:], in0=prob[:],
                                        scalar1=mg[:], scalar2=-BIG,
                                        op0=ALU.is_ge, op1=ALU.mult)
                g2 = smalls.tile([128, E], F32, tag="g2", name="g2")
                nc.vector.tensor_add(g2[:], prob[:], is1[:])
                m2 = stats.tile([128, 1], F32, tag="m2", name="m2")
                nc.vector.reduce_max(m2[:], g2[:], axis=AX.X)
                is2 = smalls.tile([128, E], F32, tag="is2", name="is2")
                nc.vector.tensor_scalar(out=is2[:], in0=prob[:],
                                        scalar1=m2[:], scalar2=None,
                                        op0=ALU.is_ge)
                nc.vector.tensor_mul(comb_b[:, j, :], prob[:], is2[:])

            # dense experts for this token block
            mix_b = mixp.tile([128, BLK // 128, D], F32, tag="mix",
                              name="mix_b")
            for e in range(E_LOC):
                ehT = ehp.tile([128, KH, BLK], BF16, tag="ehT", name="ehT")
                for m in range(KH):
                    ps1 = ps_eh.tile([128, BLK], F32, tag="eh", name="ps1")
                    for k in range(KD):
                        nc.tensor.matmul(ps1[:],
                                         ew1_t[:, e, k, m * 128:(m + 1) * 128],
                                         hT_b[:, k, :],
                                         start=(k == 0), stop=(k == KD - 1))
                    nc.scalar.activation(out=ehT[:, m, :], in_=ps1[:],
                                         func=AF.Silu,
                                         bias=eb1_t[:, e, m:m + 1])
                for ms in range(BLK // 128):
                    ps2 = ps_ey.tile([128, D], F32, tag="ey", name="ps2")
                    for k in range(KH):
                        nc.tensor.matmul(ps2[:],
                                         ehT[:, k, ms * 128:(ms + 1) * 128],
                                         ew2_t[:, e, k, :],
                                         start=(k == 0), stop=(k == KH - 1))
                    t0 = eytmp.tile([128, D], F32, tag="eyt", name="t0")
                    nc.vector.tensor_add(t0[:], ps2[:], eb2_bc[:, e, :])
                    if e == 0:
                        nc.vector.tensor_scalar_mul(
                            mix_b[:, ms, :], t0[:], comb_b[:, ms, 0:1])
                    else:
                        t1 = eytmp.tile([128, D], F32, tag="eyt2", name="t1")
                        nc.vector.tensor_scalar_mul(
                            t1[:], t0[:], comb_b[:, ms, e:e + 1])
                        nc.vector.tensor_add(
                            mix_b[:, ms, :], mix_b[:, ms, :], t1[:])
            # cast f32 -> bf16 on the way out (gpsimd DMA casts)
            nc.gpsimd.dma_start(
                out=mix_dram[b * BLK:(b + 1) * BLK, :].rearrange(
                    "(ms p) d -> p ms d", p=128),
                in_=mix_b[:])

            # chunked reduce-scatter: chunk c covers blocks 2c, 2c+1
            if b % 2 == 1:
                c = b // 2
                nc.gpsimd.collective_compute(
                    "ReduceScatter", ALU.add,
                    replica_groups=[list(range(N_CORES))],
                    ins=[mix_dram[c * CHT:(c + 1) * CHT, :].opt()],
                    outs=[rs_out[c * 128:(c + 1) * 128, :].opt()],
                )

        # ============ stage 2: moe = mix + shared; ff; residual ============
        mix_sh = hTf.tile([128, MS, D], F32, tag="hTf", name="mix_sh")
        for c in range(NCH):
            nc.gpsimd.dma_start(   # bf16 -> f32 cast
                out=mix_sh[:, c, :],
                in_=rs_out[c * 128:(c + 1) * 128, :])
        moe = mixp.tile([128, MS, D], F32, tag="mix", name="moe")
        fhT = hT_pool.tile([128, KD, TSH], BF16, tag="hT", name="fhT")
        for ms in range(MS):
            nc.vector.tensor_add(moe[:, ms, :], shared_nat[:, ms, :],
                                 mix_sh[:, ms, :])
            h_t = layernorm_tile(moe[:, ms, :], fg_bc, fb_bc, "_ff")
            transpose_to(h_t, [(fhT[:, :, ms * 128:(ms + 1) * 128], "act")])

        # ff mm1: ffhT [DH, TSH]
        fw1_t = ehp.tile([128, KD, DH], BF16, tag="ehT", name="fw1_t")
        nc.gpsimd.dma_start(
            out=fw1_t[:],
            in_=bass.AP(tensor=fw1, offset=0,
                        ap=[[DH, 128], [128 * DH, KD], [1, DH]]))
        ffhT = ehp.tile([128, KH, TSH], BF16, tag="ehT", name="ffhT")
        for m in range(KH):
            ps1 = ps_eh.tile([128, TSH], F32, tag="eh", name="ps1f")
            for k in range(KD):
                nc.tensor.matmul(ps1[:], fw1_t[:, k, m * 128:(m + 1) * 128],
                                 fhT[:, k, :],
                                 start=(k == 0), stop=(k == KD - 1))
            nc.scalar.activation(out=ffhT[:, m, :], in_=ps1[:], func=AF.Silu,
                                 bias=fb1_t[:, m:m + 1])
        # ff mm2 (k-outer over 4 psum banks) -> natural; out = x + moe + ff
        ps3s = [ps_ey.tile([128, D], F32, tag="ey", name=f"ps3s{ms}")
                if ms < 2 else
                ps_gate.tile([128, D], F32, tag="gate", name=f"ps3s{ms}")
                for ms in range(MS)]
        for k in range(KH):
            w2k = s2w.tile([128, D], BF16, tag="s2wk", name="w2kf")
            nc.gpsimd.dma_start(out=w2k[:], in_=fw2[k * 128:(k + 1) * 128, :])
            for ms in range(MS):
                nc.tensor.matmul(ps3s[ms][:],
                                 ffhT[:, k, ms * 128:(ms + 1) * 128],
                                 w2k[:],
                                 start=(k == 0), stop=(k == KH - 1))
        for ms in range(MS):
            o_t = nat.tile([128, D], F32, tag="o_t", name="o_t")
            nc.vector.tensor_add(o_t[:], ps3s[ms][:], fb2_bc[:])
            nc.vector.tensor_add(o_t[:], o_t[:], moe[:, ms, :])
            nc.vector.tensor_add(o_t[:], o_t[:], x_sh[:, ms, :])
            nc.sync.dma_start(
                out=out[ms * 128:(ms + 1) * 128, :], in_=o_t[:])

    nc.compile()
    return nc


_GRAPH = None


def _get_graph():
    global _GRAPH
    if _GRAPH is None:
        _GRAPH = build_graph()
    return _GRAPH


def _make_in_maps(inputs):
    x = np.ascontiguousarray(inputs["x"].reshape(T, D), dtype=np.float32)
    rw_full = np.asarray(inputs["router_w"], dtype=np.float32)
    rb_full = np.asarray(inputs["router_b"], dtype=np.float32)
    in_maps = []
    for i in range(N_CORES):
        lo, hi = i * E_LOC, (i + 1) * E_LOC
        perm = list(range(lo, hi)) + [e for e in range(E) if not lo <= e < hi]
        x_shard = np.concatenate([x[s:t] for s, t in _shard_rows(i)], axis=0)
        m = {
            "x": x,
            "x_shard": np.ascontiguousarray(x_shard),
            "ln_g": np.asarray(inputs["ln_in_g"], np.float32),
            "ln_b": np.asarray(inputs["ln_in_b"], np.float32),
            "rw": np.ascontiguousarray(rw_full[:, perm]),
            "rb": np.ascontiguousarray(rb_full[perm]),
            "ew1": np.ascontiguousarray(inputs["ew1"][lo:hi], np.float32),
            "eb1": np.ascontiguousarray(inputs["eb1"][lo:hi], np.float32),
            "ew2": np.ascontiguousarray(inputs["ew2"][lo:hi], np.float32),
            "eb2": np.ascontiguousarray(inputs["eb2"][lo:hi], np.float32),
            "sw1": np.asarray(inputs["sw1"], np.float32),
            "sb1": np.asarray(inputs["sb1"], np.float32),
            "sw2": np.asarray(inputs["sw2"], np.float32),
            "sb2": np.asarray(inputs["sb2"], np.float32),
            "fg": np.asarray(inputs["ff_ln_g"], np.float32),
            "fb": np.asarray(inputs["ff_ln_b"], np.float32),
            "fw1": np.asarray(inputs["ff_w1"], np.float32),
            "fb1": np.asarray(inputs["ff_b1"], np.float32),
            "fw2": np.asarray(inputs["ff_w2"], np.float32),
            "fb2": np.asarray(inputs["ff_b2"], np.float32),
        }
        in_maps.append(m)
    return in_maps


def kernel_ex(trace=False, **inputs):
    nc = _get_graph()
    in_maps = _make_in_maps(inputs)
    res = run_bass_kernel_spmd(nc, in_maps, list(range(N_CORES)), trace=trace)
    full = np.empty((T, D), dtype=np.float32)
    for i in range(N_CORES):
        o = res.results[i]["out"]
        for c, (s, t) in enumerate(_shard_rows(i)):
            full[s:t] = o[c * 128:(c + 1) * 128]
    return full.reshape(B, S, D), res


def kernel(**inputs):
    out, _ = kernel_ex(**inputs)
    return out
